# revision 1
# baseline (speedup 1.0000x reference)
"""Causal self-attention (B=4, T=2048, C=1024, H=16) on 8 trn2 cores.

Sharding: batch (4-way) x head-group (2-way).  Core i handles batch i//2 and
heads [8*(i%2), 8*(i%2)+8).  Each core computes qkv projection for its head
slice, causal attention, and a partial out-projection (contraction over its
512 att columns).  Host sums the two partials per batch.

All device compute is fp32-precision via float32r matmuls (fp32 rounded to
11 mantissa bits, streamed at full PE rate).  Host pre-transposes and
pre-rounds so the device never transposes or converts anything:
  - xT       (C, T)      : x[b].T
  - wqk      (8,128,8*128): per m-tile of [wq_g; wk_g].T, k-tiles along free
  - wv       (C, 512)    : wv_g.T
  - wo       (512, C)    : w_out.T row-slice for this head group
  - masks    (128, 1280) : packed binary causal masks for the (narrowed)
                           diagonal tile pairs: [512|384|256|128] variants
  - vinit    (128, 772)  : v_sb ones/zeros init pattern
  - ones_in  (128, 128)  : all-ones lhsT for the PE sums-broadcast matmul
Layouts on chip:
  - QT/KT  [128, 4, T]   rows = head-major (hl*64+d), T on free dim
  - V      [128, 16, 772]: per t-tile, per head pair [V_e|1] + [1|0*63|V_o]
                           (odd heads use an M=128 lhsT whose leading ones
                           column lands the softmax sums on psum row 0)
  - attT   [128, 4, T]   rows = c_local = hl*64+d  (lhsT for out-proj)

Structure: phase 1 computes Q^T/K^T/V with k-outer waves (DMA overlaps
compute); phase 2 runs causal attention j-chunk-outer with two heads
interleaved (independent PE streams), st pairs sharing one 2-bank psum tile
-> single wide exp per pair, diagonal tiles width-narrowed by causality, and
softmax sums extracted via an appended ones column + normalized through a
K=1 broadcast matmul + fast reciprocal; each chunk's out-projection (phase 3)
is emitted right after the chunk so it overlaps the next chunk's attention.
"""

from contextlib import ExitStack

import numpy as np

import concourse.bass as bass
import concourse.mybir as mybir
import concourse.tile as tile
from concourse import bacc, bass_utils

B, T, C, H, HD = 4, 2048, 1024, 16, 64
HG = 2  # head groups (tensor-parallel dim)
HPG = H // HG  # 8 heads per group
OG = HPG * HD  # 512: local width of q/k/v slice
KT_C = C // 128  # 8 contraction tiles for the projections
NT = T // 128  # 16 t-tiles
NQ = T // 512  # 4 tq chunks
PAIR_W = 65 + 128  # v_sb cols per head pair: [V_e|1] + [0*63|1|V_o]

f32 = mybir.dt.float32
f32r = mybir.dt.float32r

TRACE = False  # test.py flips this for profiling runs
DEBUG = False  # adds intermediate dumps (qt/kt/v/attT) as extra outputs
LAST_RUN = {}

_NC_CACHE = []


def _mm(nc, out, lhsT, rhs, **kw):
    nc.tensor.matmul(out, lhsT, rhs, **kw)


def _build_nc():
    nc = bacc.Bacc(trn_type="TRN2", target_bir_lowering=False, debug=False)
    xT = nc.dram_tensor("xT", [C, T], f32r, kind="ExternalInput").ap()
    wqk = nc.dram_tensor("wqk", [8, 128, 1024], f32r, kind="ExternalInput").ap()
    wv = nc.dram_tensor("wv", [C, OG], f32r, kind="ExternalInput").ap()
    wo = nc.dram_tensor("wo", [OG, C], f32r, kind="ExternalInput").ap()
    masks = nc.dram_tensor("masks", [128, 1280], f32r, kind="ExternalInput").ap()
    vinit = nc.dram_tensor("vinit", [128, 4 * PAIR_W], f32r, kind="ExternalInput").ap()
    ones_in = nc.dram_tensor("ones_in", [128, 128], f32r, kind="ExternalInput").ap()
    y = nc.dram_tensor("y", [T, C], f32, kind="ExternalOutput").ap()
    dbg = None
    if DEBUG:
        dbg = {
            "qt": nc.dram_tensor("dbg_qt", [128, 4, T], f32r, kind="ExternalOutput").ap(),
            "kt": nc.dram_tensor("dbg_kt", [128, 4, T], f32r, kind="ExternalOutput").ap(),
            "v": nc.dram_tensor("dbg_v", [128, NT, 4 * PAIR_W], f32r, kind="ExternalOutput").ap(),
            "attT": nc.dram_tensor("dbg_attT", [128, 4, T], f32r, kind="ExternalOutput").ap(),
        }

    with tile.TileContext(nc) as tc:
        _body(tc, nc, xT, wqk, wv, wo, masks, vinit, ones_in, y, dbg)
    nc.compile()
    return nc


def _body(tc, nc, xT, wqk, wv, wo, masks, vinit, ones_in, y, dbg):
    exp_f = mybir.ActivationFunctionType.Exp

    # ---- persistent tensors (allocated below the per-phase pools) ----
    with tc.tile_pool(name="persist", bufs=1) as persist:
        qt = persist.tile([128, 4, T], f32r)
        kt = persist.tile([128, 4, T], f32r)
        v_sb = persist.tile([128, NT, 4 * PAIR_W], f32r)


        # ================= phase 1: qkv projections =================
        # k-outer waves: each k-step of the contraction only needs xT k-slice
        # k, so matmuls start as soon as the first 512KB DMA lands and the
        # remaining loads overlap compute.  Waves: A = q m-tiles, B = k
        # m-tiles, V.  Each wave holds 8 psum banks.
        with (
            tc.tile_pool(name="wv_p", bufs=1) as wv_p,
            tc.tile_pool(name="xh_p", bufs=1) as xh_p,
            tc.tile_pool(name="wqk_p", bufs=1) as wqk_p,
            tc.tile_pool(name="p1ps", bufs=8, space="PSUM") as p1ps,
        ):
            wts = {}
            for half in range(2):
                t0 = half * (T // 2)
                xhk = []
                for k in range(KT_C):
                    xt = xh_p.tile(
                        [128, T // 2], f32r, tag=f"xh{k}", name=f"xh{half}_{k}"
                    )
                    xhk.append(xt)

                def load_xh(k):
                    nc.sync.dma_start(
                        xhk[k][:], xT[k * 128 : (k + 1) * 128, t0 : t0 + T // 2]
                    )

                if half == 0:
                    # priority order: first k-slice + first-wave weights, then
                    # the rest, then V weights and v_sb init pattern.
                    load_xh(0)
                    for m in range(4):
                        wt = wqk_p.tile([128, 1024], f32r, tag=f"wqk{m}", name=f"wt{m}")
                        nc.sync.dma_start(wt[:], wqk[m, :, :])
                        wts[m] = wt
                    for k in range(1, KT_C):
                        load_xh(k)
                    for m in range(4, 8):
                        wt = wqk_p.tile([128, 1024], f32r, tag=f"wqk{m}", name=f"wt{m}")
                        nc.sync.dma_start(wt[:], wqk[m, :, :])
                        wts[m] = wt
                    wv_sb = wv_p.tile([128, KT_C, OG], f32r)
                    for k in range(KT_C):
                        nc.sync.dma_start(
                            wv_sb[:, k, :], wv[k * 128 : (k + 1) * 128, :]
                        )
                    for tt in range(NT):
                        nc.sync.dma_start(v_sb[:, tt, :], vinit[:])
                else:
                    for k in range(KT_C):
                        load_xh(k)

                for wave in range(2):  # A: q (m 0..3), B: k (m 4..7)
                    dst = qt if wave == 0 else kt
                    pss = [
                        p1ps.tile([128, 512], f32, tag="p1", name=f"p1_{half}_{wave}_{i}")
                        for i in range(8)
                    ]
                    for k in range(KT_C):
                        for mi in range(4):
                            wt = wts[wave * 4 + mi]
                            for n in range(2):
                                _mm(
                                    nc,
                                    pss[mi * 2 + n][:],
                                    wt[:, k * 128 : (k + 1) * 128],
                                    xhk[k][:, n * 512 : (n + 1) * 512],
                                    start=(k == 0),
                                    stop=(k == KT_C - 1),
                                )
                    for mi in range(4):
                        for n in range(2):
                            nc.scalar.copy(
                                dst[
                                    :,
                                    mi,
                                    t0 + n * 512 : t0 + (n + 1) * 512,
                                ],
                                pss[mi * 2 + n][:],
                            )
                # V wave: out rows t, free = o (head-major)
                psv = [
                    p1ps.tile([128, 512], f32, tag="p1", name=f"p1v_{half}_{i}")
                    for i in range(8)
                ]
                for k in range(KT_C):
                    for tl in range(8):
                        _mm(
                            nc,
                            psv[tl][:],
                            xhk[k][:, tl * 128 : (tl + 1) * 128],
                            wv_sb[:, k, :],
                            start=(k == 0),
                            stop=(k == KT_C - 1),
                        )
                for tl in range(8):
                    tt = half * 8 + tl
                    ps = psv[tl]
                    src_e = ps[:].rearrange("p (h d) -> p h d", d=64)[:, 0::2, :]
                    src_o = ps[:].rearrange("p (h d) -> p h d", d=64)[:, 1::2, :]
                    dstv = v_sb[:, tt, :].rearrange("p (q w) -> p q w", w=PAIR_W)
                    nc.vector.tensor_copy(dstv[:, :, 0:64], src_e)
                    nc.vector.tensor_copy(dstv[:, :, 129:193], src_o)

        if dbg is not None:
            for mm_ in range(4):
                nc.sync.dma_start(dbg["qt"][:, mm_, :], qt[:, mm_, :])
                nc.sync.dma_start(dbg["kt"][:, mm_, :], kt[:, mm_, :])
            for tt_ in range(NT):
                nc.sync.dma_start(dbg["v"][:, tt_, :], v_sb[:, tt_, :])

        # ================= phase 2: attention =================
        # attT opens after phase-1 pools close so the stack allocator reuses
        # their SBUF; it stays open through phase 3 (closed at the end).
        att_ctx = ExitStack()
        attp = att_ctx.enter_context(tc.tile_pool(name="attp", bufs=1))
        attT = attp.tile([128, 4, T], f32r)
        with (
            tc.tile_pool(name="mask_p", bufs=1) as mask_p,
            tc.tile_pool(name="pt_p", bufs=3) as pt_p,
            tc.tile_pool(name="recip_p", bufs=1) as recip_p,
            tc.tile_pool(name="bcast_p", bufs=1) as bcast_p,
            tc.tile_pool(name="ones_p", bufs=1) as ones_p,
            tc.tile_pool(name="wo_p", bufs=1) as wo_p,
            tc.tile_pool(name="yo_p", bufs=2) as yo_p,
            tc.tile_pool(name="y_ps", bufs=1, space="PSUM") as y_ps,
            tc.tile_pool(name="st_ps", bufs=2, space="PSUM") as st_ps,
            tc.tile_pool(name="av_ps", bufs=2, space="PSUM") as av_ps,
        ):
            mk = mask_p.tile([128, 1280], f32r)
            nc.sync.dma_start(mk[:], masks[:])
            ones_sb = ones_p.tile([128, 128], f32r)
            nc.sync.dma_start(ones_sb[:], ones_in[:])
            wo_sb = wo_p.tile([128, 4, C], f32r)
            for k in range(4):
                nc.sync.dma_start(wo_sb[:, k, :], wo[k * 128 : (k + 1) * 128, :])

            def head_ctx(hl):
                """Slices/layout facts for local head hl."""
                p0 = (hl % 2) * 64
                mt = hl // 2
                qrow = slice(p0, p0 + 64)
                vb0 = (hl // 2) * PAIR_W
                if hl % 2 == 0:
                    vsl = (vb0, vb0 + 65)  # [V|1] -> rows 0..64
                    srow, arow = 64, slice(0, 64)
                else:
                    vsl = (vb0 + 65, vb0 + 193)  # [1|0*63|V] -> row 0 sums, 64..127 att
                    srow, arow = 0, slice(64, 128)
                return p0, mt, qrow, vsl, srow, arow

            # j-outer: all head pairs for one tq chunk, then that chunk's
            # out-projection - phase 3 work overlaps the next chunk's attention.
            def do_chunk(j):
                ntk = 4 * j + 4
                ng = ntk // 2
                tq = slice(j * 512, (j + 1) * 512)
                for ha in range(0, HPG, 2):
                    ctxs = [head_ctx(ha), head_ctx(ha + 1)]
                    pts = {0: [None] * ng, 1: [None] * ng}

                    def emit_pair(s, g):
                        _, mt, qrow, _, _, _ = ctxs[s]
                        # diagonal tiles only need tq >= tk: narrow the
                        # st/exp/av width (512/384/256/128) instead of masking
                        # fully-computed tiles.
                        geom = []  # per u: (tq_off, width, pt_col)
                        pcol = 0
                        for u in range(2):
                            tk = 2 * g + u
                            v = tk - 4 * j
                            off = 128 * v if v > 0 else 0
                            w = 512 - off
                            if u == 1 and pcol == 512:
                                pcol = 512  # second slot starts at bank 1
                            geom.append((off, w, pcol))
                            pcol = 512 if u == 0 and w == 512 else pcol + w
                        dg = 2 * g - 4 * j
                        ps = st_ps.tile([128, 1024], f32, tag="st")
                        for u in range(2):
                            off, w, pc = geom[u]
                            tk = 2 * g + u
                            _mm(
                                nc,
                                ps[:, pc : pc + w],
                                kt[qrow, mt, tk * 128 : (tk + 1) * 128],
                                qt[qrow, mt, j * 512 + off : (j + 1) * 512],
                                start=True,
                                stop=True,
                            )
                        tot = geom[1][2] + geom[1][1]
                        pt = pt_p.tile([128, 1024], f32r, tag=f"pt{s}")
                        nc.scalar.activation(
                            pt[:, 0:tot], ps[:, 0:tot], exp_f, scale=0.125
                        )
                        if dg == 0:  # pair (4j, 4j+1): widths 512|384
                            nc.vector.tensor_mul(
                                pt[:, 0:896], pt[:, 0:896], mk[:, 0:896]
                            )
                        elif dg == 2:  # pair (4j+2, 4j+3): widths 256|128
                            nc.vector.tensor_mul(
                                pt[:, 0:384], pt[:, 0:384], mk[:, 896:1280]
                            )
                        pts[s][g] = (pt, geom)

                    avs = [
                        av_ps.tile([128, 512], f32, tag="av", name=f"av{s}_{ha}_{j}")
                        for s in (0, 1)
                    ]
                    emit_pair(0, 0)
                    emit_pair(1, 0)
                    for g in range(ng):
                        if g + 1 < ng:
                            emit_pair(0, g + 1)
                            emit_pair(1, g + 1)
                        for u in range(2):
                            for s in (0, 1):
                                _, _, _, vsl, _, _ = ctxs[s]
                                pt, geom = pts[s][g]
                                off, w, pc = geom[u]
                                tk = 2 * g + u
                                _mm(
                                    nc,
                                    avs[s][0 : vsl[1] - vsl[0], off : off + w],
                                    v_sb[:, tk, vsl[0] : vsl[1]],
                                    pt[:, pc : pc + w],
                                    start=(tk == 0),
                                    stop=(tk == ntk - 1),
                                )

                    for s in (0, 1):
                        _, mt, _, _, srow, arow = ctxs[s]
                        av = avs[s]
                        sums_sb = recip_p.tile([128, 512], f32r, tag=f"rc{s}")
                        nc.scalar.copy(
                            sums_sb[srow : srow + 1, :], av[srow : srow + 1, :]
                        )
                        bps = st_ps.tile(
                            [128, 1024], f32, tag="st", name=f"bps_{ha}_{j}_{s}"
                        )
                        _mm(
                            nc,
                            bps[:, 0:512],
                            ones_sb[srow : srow + 1, :],
                            sums_sb[srow : srow + 1, :],
                            start=True,
                            stop=True,
                        )
                        bc = bcast_p.tile([128, 512], f32, tag=f"bc{s}")
                        nc.vector.reciprocal_approx_fast(bc[:], bps[:, 0:512])
                        nc.vector.tensor_mul(
                            attT[arow, mt, tq], av[arow, :], bc[arow, :]
                        )

            def do_outproj_chunk(j):
                # y rows for tq chunk j: 4 t-tiles x 2 o-halves
                for tl in range(4):
                    tt = 4 * j + tl
                    yps = y_ps.tile([128, 1024], f32, tag="y", name=f"yps_{tt}")
                    pso = [yps[:, 0:512], yps[:, 512:1024]]
                    for k in range(4):
                        for o in range(2):
                            _mm(
                                nc,
                                pso[o],
                                attT[:, k, tt * 128 : (tt + 1) * 128],
                                wo_sb[:, k, o * 512 : (o + 1) * 512],
                                start=(k == 0),
                                stop=(k == 3),
                            )
                    for o in range(2):
                        yo = yo_p.tile([128, 512], f32, tag="yo", name=f"yo_{tt}_{o}")
                        nc.scalar.copy(yo[:], pso[o])
                        nc.sync.dma_start(
                            y[tt * 128 : (tt + 1) * 128, o * 512 : (o + 1) * 512],
                            yo[:],
                        )

            for j in (3, 2, 1, 0):
                do_chunk(j)
                do_outproj_chunk(j)

        if dbg is not None:
            for mm_ in range(4):
                nc.sync.dma_start(dbg["attT"][:, mm_, :], attT[:, mm_, :])

        att_ctx.close()


def _round_fp32r(a):
    """Round fp32 to the fp32r grid (11 mantissa bits; low 12 bits zero), RNE."""
    u = np.ascontiguousarray(a, dtype=np.float32).view(np.uint32)
    lsb = (u >> 12) & 1
    out = ((u + 0x7FF + lsb) & 0xFFFFF000).astype(np.uint32)
    return out.view(np.float32)


def _host_prep(x, w_qkv, w_out):
    xT_all = np.ascontiguousarray(x.transpose(0, 2, 1)).astype(np.float32)
    # packed diagonal masks, all variant-0 (keep iff tq_local >= tk_local):
    # [0:512) pair1-u0 w=512, [512:896) pair1-u1 w=384,
    # [896:1152) pair2-u0 w=256, [1152:1280) pair2-u1 w=128
    tk_l = np.arange(128)[:, None]
    m0 = (np.arange(512)[None, :] >= tk_l).astype(np.float32)
    masks = np.concatenate([m0, m0[:, :384], m0[:, :256], m0[:, :128]], axis=1)

    per_group = []
    for g in range(HG):
        wq = w_qkv[g * OG : (g + 1) * OG]
        wk = w_qkv[C + g * OG : C + (g + 1) * OG]
        wvg = w_qkv[2 * C + g * OG : 2 * C + (g + 1) * OG]
        wqkT = np.concatenate([wq, wk], axis=0).T  # (C, 1024)
        # wqk_r[m, p, k*128+j] = wqkT[k*128+p, m*128+j]
        wqk_r = np.ascontiguousarray(
            wqkT.reshape(8, 128, 8, 128).transpose(2, 1, 0, 3).reshape(8, 128, 1024)
        ).astype(np.float32)
        wv_t = np.ascontiguousarray(wvg.T).astype(np.float32)  # (C, 512)
        wo_t = np.ascontiguousarray(w_out.T[g * OG : (g + 1) * OG]).astype(
            np.float32
        )  # (512, C)
        per_group.append((_round_fp32r(wqk_r), _round_fp32r(wv_t), _round_fp32r(wo_t)))
    vinit = np.zeros((128, 4 * PAIR_W), np.float32)
    for pr in range(4):
        vinit[:, pr * PAIR_W + 64] = 1.0  # even-head ones col
        vinit[:, pr * PAIR_W + 65] = 1.0  # odd-head ones col (block col 0)
    ones_in = np.ones((128, 128), np.float32)
    return _round_fp32r(xT_all), masks, vinit, ones_in, per_group


def kernel(x, w_qkv, w_out):
    x = np.asarray(x)
    w_qkv = np.asarray(w_qkv)
    w_out = np.asarray(w_out)
    xT_all, masks, vinit, ones_in, per_group = _host_prep(x, w_qkv, w_out)

    if not _NC_CACHE:
        _NC_CACHE.append(_build_nc())
    nc = _NC_CACHE[0]

    in_maps = []
    for core in range(8):
        b, g = core // 2, core % 2
        wqk_r, wv_t, wo_t = per_group[g]
        in_maps.append(
            {"xT": xT_all[b], "wqk": wqk_r, "wv": wv_t, "wo": wo_t, "masks": masks,
             "vinit": vinit, "ones_in": ones_in}
        )

    res = bass_utils.run_bass_kernel_spmd(
        nc, in_maps, core_ids=list(range(8)), trace=TRACE
    )
    LAST_RUN["res"] = res

    y = np.empty((B, T, C), np.float32)
    for b in range(B):
        y[b] = res.results[2 * b]["y"] + res.results[2 * b + 1]["y"]
    return y



# revision 2
# speedup vs baseline: 1.3347x; 1.3347x over previous
"""Causal self-attention (B=4, T=2048, C=1024, H=16) on 8 trn2 cores.

Sharding: batch (4-way) x head-group (2-way).  Core i handles batch i//2 and
heads [8*(i%2), 8*(i%2)+8).  Each core computes qkv projection for its head
slice, causal attention, and a partial out-projection (contraction over its
512 att columns).  Host sums the two partials per batch.

v2 (merged-phase schedule): the PE HAM clock-gate throttles to 1.2 GHz
whenever the tensor engine micro-idles, so the kernel is structured to keep
the PE stream dense end-to-end:
  - projections and attention interleave: half-0 projections, then chunk 0/1
    attention (which only needs t<1024 of K/V), then half-1 projections, then
    chunks 2/3.  The tile scheduler fills PE gaps (waiting on exp) with
    projection matmuls and vice versa.
  - PSUM is partitioned so both phases coexist: one shared 3-slot pool of
    2-bank tiles (projection sub-waves, score tiles, out-proj accumulators,
    sums-broadcast) + 2 single-bank AV accumulators.
  - ACT (scalar engine) does exp + the tiny sums-row copies only; all other
    psum evacuation (q/k/v projections, y tiles) runs on DVE.
  - everything downstream of the projections is bf16 (same 1 cycle/row PE
    stream rate, half the SBUF, 2x DVE on sbuf-sbuf ops): qt/kt/v_sb/attT/
    wo/masks/pt.  Projection inputs stay fp32r.

Layouts on chip (same as v1):
  - QT/KT  [128, 4, T]   rows = head-major (hl*64+d), T on free dim
  - V      [128, 16, 772]: per t-tile, per head pair [V_e|1] + [1|0*63|V_o]
  - attT   [128, 4, T]   rows = c_local = hl*64+d  (lhsT for out-proj)
Softmax sums come from the appended ones columns in V (even head: psum row
64; odd head: row 0), then ACT copy -> PE ones-broadcast -> DVE reciprocal
-> DVE normalize into attT.
"""

from contextlib import ExitStack

import numpy as np
import ml_dtypes

import concourse.bass as bass
import concourse.mybir as mybir
import concourse.tile as tile
from concourse import bacc, bass_utils

B, T, C, H, HD = 4, 2048, 1024, 16, 64
HG = 2  # head groups (tensor-parallel dim)
HPG = H // HG  # 8 heads per group
OG = HPG * HD  # 512: local width of q/k/v slice
KT_C = C // 128  # 8 contraction tiles for the projections
NT = T // 128  # 16 t-tiles
NQ = T // 512  # 4 tq chunks
PAIR_W = 65 + 128  # v_sb cols per head pair: [V_e|1] + [0*63|1|V_o]

f32 = mybir.dt.float32
f32r = mybir.dt.float32r
bf16 = mybir.dt.bfloat16
BF16 = ml_dtypes.bfloat16

TRACE = False  # test.py flips this for profiling runs
DEBUG = False  # adds intermediate dumps (qt/kt/v/attT) as extra outputs
LAST_RUN = {}

_NC_CACHE = []


def _mm(nc, out, lhsT, rhs, **kw):
    nc.tensor.matmul(out, lhsT, rhs, **kw)


def _build_nc():
    nc = bacc.Bacc(trn_type="TRN2", target_bir_lowering=False, debug=False)
    xT = nc.dram_tensor("xT", [C, T], f32r, kind="ExternalInput").ap()
    wqk = nc.dram_tensor("wqk", [8, 128, 1024], f32r, kind="ExternalInput").ap()
    wv = nc.dram_tensor("wv", [C, OG], f32r, kind="ExternalInput").ap()
    wo = nc.dram_tensor("wo", [OG, C], bf16, kind="ExternalInput").ap()
    masks = nc.dram_tensor("masks", [128, 1280], bf16, kind="ExternalInput").ap()
    vinit = nc.dram_tensor("vinit", [128, 4 * PAIR_W], bf16, kind="ExternalInput").ap()
    ones_in = nc.dram_tensor("ones_in", [128, 128], f32r, kind="ExternalInput").ap()
    y = nc.dram_tensor("y", [T, C], f32, kind="ExternalOutput").ap()
    dbg = None
    if DEBUG:
        dbg = {
            "qt": nc.dram_tensor("dbg_qt", [128, 4, T], bf16, kind="ExternalOutput").ap(),
            "kt": nc.dram_tensor("dbg_kt", [128, 4, T], bf16, kind="ExternalOutput").ap(),
            "v": nc.dram_tensor("dbg_v", [128, NT, 4 * PAIR_W], bf16, kind="ExternalOutput").ap(),
            "attT": nc.dram_tensor("dbg_attT", [128, 4, T], bf16, kind="ExternalOutput").ap(),
        }

    with tile.TileContext(nc) as tc:
        _body(tc, nc, xT, wqk, wv, wo, masks, vinit, ones_in, y, dbg)
    nc.compile()
    return nc


def _body(tc, nc, xT, wqk, wv, wo, masks, vinit, ones_in, y, dbg):
    exp_f = mybir.ActivationFunctionType.Exp

    with (
        tc.tile_pool(name="persist", bufs=1) as persist,
        tc.tile_pool(name="wv_p", bufs=1) as wv_p,
        tc.tile_pool(name="xh_p", bufs=1) as xh_p,
        tc.tile_pool(name="wqk_p", bufs=1) as wqk_p,
        tc.tile_pool(name="mask_p", bufs=1) as mask_p,
        tc.tile_pool(name="ones_p", bufs=1) as ones_p,
        tc.tile_pool(name="wo_p", bufs=1) as wo_p,
        tc.tile_pool(name="pt_p", bufs=3) as pt_p,
        tc.tile_pool(name="sums_p", bufs=1) as sums_p,
        tc.tile_pool(name="bcast_p", bufs=1) as bcast_p,
        tc.tile_pool(name="yo_p", bufs=2) as yo_p,
        tc.tile_pool(name="big_ps", bufs=3, space="PSUM") as big_ps,
        tc.tile_pool(name="av_ps", bufs=2, space="PSUM") as av_ps,
    ):
        qt = persist.tile([128, 4, T], bf16)
        kt = persist.tile([128, 4, T], bf16)
        v_sb = persist.tile([128, NT, 4 * PAIR_W], bf16)
        attT = persist.tile([128, 4, T], bf16)

        mk = mask_p.tile([128, 1280], bf16)
        ones_sb = ones_p.tile([128, 128], f32r)
        wo_sb = wo_p.tile([128, 4, C], bf16)
        wv_sb = wv_p.tile([128, KT_C, OG], f32r)
        wts = {}

        # ================= projections (one half of T) =================
        # k-outer waves: each k-step of the contraction only needs xT k-slice
        # k, so matmuls start as soon as the first DMA lands.  Sub-waves of
        # two 2-bank psum tiles keep the shared "big" pool free for the
        # attention chunks that interleave with the second half.
        def emit_half(half):
            t0 = half * (T // 2)
            xs = []
            for k in range(KT_C):
                xt = xh_p.tile(
                    [128, T // 2], f32r, tag=f"xh{k}", name=f"xh{half}_{k}"
                )
                xs.append(xt)

            def load_xh(k):
                nc.sync.dma_start(
                    xs[k][:], xT[k * 128 : (k + 1) * 128, t0 : t0 + T // 2]
                )

            if half == 0:
                # priority order: first k-slice + first-wave weights, then
                # the rest, then V weights / v_sb init / consts.
                load_xh(0)
                for m in range(4):
                    wt = wqk_p.tile([128, 1024], f32r, tag=f"wqk{m}", name=f"wt{m}")
                    nc.sync.dma_start(wt[:], wqk[m, :, :])
                    wts[m] = wt
                for k in range(1, KT_C):
                    load_xh(k)
                for m in range(4, 8):
                    wt = wqk_p.tile([128, 1024], f32r, tag=f"wqk{m}", name=f"wt{m}")
                    nc.sync.dma_start(wt[:], wqk[m, :, :])
                    wts[m] = wt
                for k in range(KT_C):
                    nc.sync.dma_start(wv_sb[:, k, :], wv[k * 128 : (k + 1) * 128, :])
                for tt in range(NT):
                    nc.sync.dma_start(v_sb[:, tt, :], vinit[:])
                nc.sync.dma_start(mk[:], masks[:])
                nc.sync.dma_start(ones_sb[:], ones_in[:])
                for k in range(4):
                    nc.sync.dma_start(wo_sb[:, k, :], wo[k * 128 : (k + 1) * 128, :])
            else:
                for k in range(KT_C):
                    load_xh(k)

            for wave in range(2):  # A: q (m 0..3), B: k (m 4..7)
                dst = qt if wave == 0 else kt
                for sub in range(2):
                    bigs = [
                        big_ps.tile(
                            [128, 1024], f32, tag="big",
                            name=f"pw{half}_{wave}_{sub}_{i}",
                        )
                        for i in range(2)
                    ]
                    for k in range(KT_C):
                        for mi2 in range(2):
                            m = wave * 4 + sub * 2 + mi2
                            for n in range(2):
                                _mm(
                                    nc,
                                    bigs[mi2][:, n * 512 : (n + 1) * 512],
                                    wts[m][:, k * 128 : (k + 1) * 128],
                                    xs[k][:, n * 512 : (n + 1) * 512],
                                    start=(k == 0),
                                    stop=(k == KT_C - 1),
                                )
                    for mi2 in range(2):
                        ml = sub * 2 + mi2
                        for n in range(2):
                            nc.vector.tensor_copy(
                                dst[:, ml, t0 + n * 512 : t0 + (n + 1) * 512],
                                bigs[mi2][:, n * 512 : (n + 1) * 512],
                            )

            # V wave: out rows t, free = o (head-major)
            for sub in range(2):
                bigs = [
                    big_ps.tile(
                        [128, 1024], f32, tag="big", name=f"pv{half}_{sub}_{i}"
                    )
                    for i in range(2)
                ]
                for k in range(KT_C):
                    for tli in range(4):
                        tl = sub * 4 + tli
                        _mm(
                            nc,
                            bigs[tli // 2][:, (tli % 2) * 512 : (tli % 2 + 1) * 512],
                            xs[k][:, tl * 128 : (tl + 1) * 128],
                            wv_sb[:, k, :],
                            start=(k == 0),
                            stop=(k == KT_C - 1),
                        )
                for tli in range(4):
                    tl = sub * 4 + tli
                    tt = half * 8 + tl
                    ps = bigs[tli // 2][:, (tli % 2) * 512 : (tli % 2 + 1) * 512]
                    src = ps.rearrange("p (h d) -> p h d", d=64)
                    dstv = v_sb[:, tt, :].rearrange("p (q w) -> p q w", w=PAIR_W)
                    nc.vector.tensor_copy(dstv[:, :, 0:64], src[:, 0::2, :])
                    nc.vector.tensor_copy(dstv[:, :, 129:193], src[:, 1::2, :])

        # ================= attention =================
        def head_ctx(hl):
            """Slices/layout facts for local head hl."""
            p0 = (hl % 2) * 64
            mt = hl // 2
            qrow = slice(p0, p0 + 64)
            vb0 = (hl // 2) * PAIR_W
            if hl % 2 == 0:
                vsl = (vb0, vb0 + 65)  # [V|1] -> rows 0..64
                srow, arow = 64, slice(0, 64)
            else:
                vsl = (vb0 + 65, vb0 + 193)  # [1|0*63|V] -> row 0 sums, 64..127 att
                srow, arow = 0, slice(64, 128)
            return p0, mt, qrow, vsl, srow, arow

        def do_chunk(j):
            ntk = 4 * j + 4
            ng = ntk // 2
            tq = slice(j * 512, (j + 1) * 512)
            for ha in range(0, HPG, 2):
                ctxs = [head_ctx(ha), head_ctx(ha + 1)]
                pts = {0: [None] * ng, 1: [None] * ng}

                def emit_pair(s, g):
                    _, mt, qrow, _, _, _ = ctxs[s]
                    # diagonal tiles only need tq >= tk: narrow the
                    # st/exp/av width (512/384/256/128) instead of masking
                    # fully-computed tiles.
                    geom = []  # per u: (tq_off, width, pt_col)
                    pcol = 0
                    for u in range(2):
                        tk = 2 * g + u
                        v = tk - 4 * j
                        off = 128 * v if v > 0 else 0
                        w = 512 - off
                        if u == 1 and pcol == 512:
                            pcol = 512  # second slot starts at bank 1
                        geom.append((off, w, pcol))
                        pcol = 512 if u == 0 and w == 512 else pcol + w
                    dg = 2 * g - 4 * j
                    ps = big_ps.tile(
                        [128, 1024], f32, tag="big", name=f"st_{j}_{ha}_{s}_{g}"
                    )
                    for u in range(2):
                        off, w, pc = geom[u]
                        tk = 2 * g + u
                        _mm(
                            nc,
                            ps[:, pc : pc + w],
                            kt[qrow, mt, tk * 128 : (tk + 1) * 128],
                            qt[qrow, mt, j * 512 + off : (j + 1) * 512],
                            start=True,
                            stop=True,
                        )
                    tot = geom[1][2] + geom[1][1]
                    pt = pt_p.tile([128, 1024], bf16, tag=f"pt{s}")
                    nc.scalar.activation(
                        pt[:, 0:tot], ps[:, 0:tot], exp_f, scale=0.125
                    )
                    if dg == 0:  # pair (4j, 4j+1): widths 512|384
                        nc.vector.tensor_mul(
                            pt[:, 0:896], pt[:, 0:896], mk[:, 0:896]
                        )
                    elif dg == 2:  # pair (4j+2, 4j+3): widths 256|128
                        nc.vector.tensor_mul(
                            pt[:, 0:384], pt[:, 0:384], mk[:, 896:1280]
                        )
                    pts[s][g] = (pt, geom)

                avs = [
                    av_ps.tile([128, 512], f32, tag="av", name=f"av{s}_{ha}_{j}")
                    for s in (0, 1)
                ]
                emit_pair(0, 0)
                emit_pair(1, 0)
                for g in range(ng):
                    if g + 1 < ng:
                        emit_pair(0, g + 1)
                        emit_pair(1, g + 1)
                    for u in range(2):
                        for s in (0, 1):
                            _, _, _, vsl, _, _ = ctxs[s]
                            pt, geom = pts[s][g]
                            off, w, pc = geom[u]
                            tk = 2 * g + u
                            _mm(
                                nc,
                                avs[s][0 : vsl[1] - vsl[0], off : off + w],
                                v_sb[:, tk, vsl[0] : vsl[1]],
                                pt[:, pc : pc + w],
                                start=(tk == 0),
                                stop=(tk == ntk - 1),
                            )

                for s in (0, 1):
                    _, mt, _, _, srow, arow = ctxs[s]
                    av = avs[s]
                    sums_sb = sums_p.tile([128, 512], f32r, tag=f"rc{s}")
                    nc.scalar.copy(
                        sums_sb[srow : srow + 1, :], av[srow : srow + 1, :]
                    )
                    bps = big_ps.tile(
                        [128, 1024], f32, tag="big", name=f"bps_{ha}_{j}_{s}"
                    )
                    _mm(
                        nc,
                        bps[:, 0:512],
                        ones_sb[srow : srow + 1, :],
                        sums_sb[srow : srow + 1, :],
                        start=True,
                        stop=True,
                    )
                    bc = bcast_p.tile([128, 512], f32, tag=f"bc{s}")
                    nc.vector.reciprocal_approx_fast(bc[:], bps[:, 0:512])
                    nc.vector.tensor_mul(
                        attT[arow, mt, tq], av[arow, :], bc[arow, :]
                    )

        def do_outproj_chunk(j):
            # y rows for tq chunk j: 4 t-tiles x 2 o-halves
            for tl in range(4):
                tt = 4 * j + tl
                yps = big_ps.tile([128, 1024], f32, tag="big", name=f"yps_{tt}")
                pso = [yps[:, 0:512], yps[:, 512:1024]]
                for k in range(4):
                    for o in range(2):
                        _mm(
                            nc,
                            pso[o],
                            attT[:, k, tt * 128 : (tt + 1) * 128],
                            wo_sb[:, k, o * 512 : (o + 1) * 512],
                            start=(k == 0),
                            stop=(k == 3),
                        )
                for o in range(2):
                    yo = yo_p.tile([128, 512], f32, tag="yo", name=f"yo_{tt}_{o}")
                    nc.vector.tensor_copy(yo[:], pso[o])
                    nc.sync.dma_start(
                        y[tt * 128 : (tt + 1) * 128, o * 512 : (o + 1) * 512],
                        yo[:],
                    )

        # merged schedule: chunks 0/1 only need t<1024 of K/V, so they ride
        # between the two projection halves and their exp/AV work overlaps
        # the half-1 projection matmuls (and vice versa).
        emit_half(0)
        do_chunk(0)
        do_outproj_chunk(0)
        do_chunk(1)
        emit_half(1)
        do_outproj_chunk(1)
        do_chunk(2)
        do_outproj_chunk(2)
        do_chunk(3)
        do_outproj_chunk(3)

        if dbg is not None:
            for mm_ in range(4):
                nc.sync.dma_start(dbg["qt"][:, mm_, :], qt[:, mm_, :])
                nc.sync.dma_start(dbg["kt"][:, mm_, :], kt[:, mm_, :])
                nc.sync.dma_start(dbg["attT"][:, mm_, :], attT[:, mm_, :])
            for tt_ in range(NT):
                nc.sync.dma_start(dbg["v"][:, tt_, :], v_sb[:, tt_, :])


def _round_fp32r(a):
    """Round fp32 to the fp32r grid (11 mantissa bits; low 12 bits zero), RNE."""
    u = np.ascontiguousarray(a, dtype=np.float32).view(np.uint32)
    lsb = (u >> 12) & 1
    out = ((u + 0x7FF + lsb) & 0xFFFFF000).astype(np.uint32)
    return out.view(np.float32)


def _host_prep(x, w_qkv, w_out):
    xT_all = np.ascontiguousarray(x.transpose(0, 2, 1)).astype(np.float32)
    # packed diagonal masks, all variant-0 (keep iff tq_local >= tk_local):
    # [0:512) pair1-u0 w=512, [512:896) pair1-u1 w=384,
    # [896:1152) pair2-u0 w=256, [1152:1280) pair2-u1 w=128
    tk_l = np.arange(128)[:, None]
    m0 = (np.arange(512)[None, :] >= tk_l).astype(BF16)
    masks = np.concatenate([m0, m0[:, :384], m0[:, :256], m0[:, :128]], axis=1)

    per_group = []
    for g in range(HG):
        wq = w_qkv[g * OG : (g + 1) * OG]
        wk = w_qkv[C + g * OG : C + (g + 1) * OG]
        wvg = w_qkv[2 * C + g * OG : 2 * C + (g + 1) * OG]
        wqkT = np.concatenate([wq, wk], axis=0).T  # (C, 1024)
        # wqk_r[m, p, k*128+j] = wqkT[k*128+p, m*128+j]
        wqk_r = np.ascontiguousarray(
            wqkT.reshape(8, 128, 8, 128).transpose(2, 1, 0, 3).reshape(8, 128, 1024)
        ).astype(np.float32)
        wv_t = np.ascontiguousarray(wvg.T).astype(np.float32)  # (C, 512)
        wo_t = np.ascontiguousarray(w_out.T[g * OG : (g + 1) * OG]).astype(
            BF16
        )  # (512, C)
        per_group.append((_round_fp32r(wqk_r), _round_fp32r(wv_t), wo_t))
    vinit = np.zeros((128, 4 * PAIR_W), BF16)
    for pr in range(4):
        vinit[:, pr * PAIR_W + 64] = 1.0  # even-head ones col
        vinit[:, pr * PAIR_W + 65] = 1.0  # odd-head ones col (block col 0)
    ones_in = np.ones((128, 128), np.float32)
    return _round_fp32r(xT_all), masks, vinit, ones_in, per_group


def kernel(x, w_qkv, w_out):
    x = np.asarray(x)
    w_qkv = np.asarray(w_qkv)
    w_out = np.asarray(w_out)
    xT_all, masks, vinit, ones_in, per_group = _host_prep(x, w_qkv, w_out)

    if not _NC_CACHE:
        _NC_CACHE.append(_build_nc())
    nc = _NC_CACHE[0]

    in_maps = []
    for core in range(8):
        b, g = core // 2, core % 2
        wqk_r, wv_t, wo_t = per_group[g]
        in_maps.append(
            {"xT": xT_all[b], "wqk": wqk_r, "wv": wv_t, "wo": wo_t, "masks": masks,
             "vinit": vinit, "ones_in": ones_in}
        )

    res = bass_utils.run_bass_kernel_spmd(
        nc, in_maps, core_ids=list(range(8)), trace=TRACE
    )
    LAST_RUN["res"] = res

    y = np.empty((B, T, C), np.float32)
    for b in range(B):
        y[b] = res.results[2 * b]["y"] + res.results[2 * b + 1]["y"]
    return y


# revision 4
# speedup vs baseline: 1.3369x; 1.0016x over previous
"""Causal self-attention (B=4, T=2048, C=1024, H=16) on 8 trn2 cores.

Sharding: batch (4-way) x head-group (2-way).  Core i handles batch i//2 and
heads [8*(i%2), 8*(i%2)+8).  Each core computes qkv projection for its head
slice, causal attention, and a partial out-projection (contraction over its
512 att columns).  Host sums the two partials per batch.

v3 (interleaved emission): the PE HAM clock-gate throttles to 1.2 GHz
whenever the tensor engine micro-idles, and engine queues execute in the
statically scheduled order, so independent work is interleaved at emission
granularity with generators:
  phase A: half-0 projections (serial, DMA-bound lead-in)
  phase B: chunk-0/1 attention pairs round-robined with half-1 projection
           sub-waves (attention exp/DVE latency hides under projection
           matmuls and vice versa)
  phase C: chunk-2 pairs round-robined with chunk-0/1 out-projections
  phase D: chunk-3 pairs round-robined with chunk-2 out-projections
  phase E: chunk-3 out-projection
PSUM: one shared pool of 2-bank tiles (bufs=3: score tiles / out-proj
accumulators / sums-broadcast / projection sub-waves, each holding one) + 2
single-bank AV accumulators.  ACT does exp + sums-row copies only; all other
psum evacuation is DVE.  Everything is bf16 except the f32 psum paths and
the normalization chain (projection inputs bf16 halves the startup DMA).

Layouts on chip (same as v1):
  - QT/KT  [128, 4, T]   rows = head-major (hl*64+d), T on free dim
  - V      [128, 16, 772]: per t-tile, per head pair [V_e|1] + [1|0*63|V_o]
  - attT   [128, 4, T]   rows = c_local = hl*64+d  (lhsT for out-proj)
Softmax sums come from the appended ones columns in V (even head: psum row
64; odd head: row 0), then ACT copy -> PE ones-broadcast -> DVE reciprocal
-> DVE normalize into attT.
"""

import numpy as np
import ml_dtypes

import concourse.bass as bass
import concourse.mybir as mybir
import concourse.tile as tile
from concourse import bacc, bass_utils

B, T, C, H, HD = 4, 2048, 1024, 16, 64
HG = 2  # head groups (tensor-parallel dim)
HPG = H // HG  # 8 heads per group
OG = HPG * HD  # 512: local width of q/k/v slice
KT_C = C // 128  # 8 contraction tiles for the projections
NT = T // 128  # 16 t-tiles
NQ = T // 512  # 4 tq chunks
PAIR_W = 65 + 128  # v_sb cols per head pair: [V_e|1] + [0*63|1|V_o]

f32 = mybir.dt.float32
f32r = mybir.dt.float32r
bf16 = mybir.dt.bfloat16
BF16 = ml_dtypes.bfloat16

TRACE = False  # test.py flips this for profiling runs
DEBUG = False  # adds intermediate dumps (qt/kt/v/attT) as extra outputs
LAST_RUN = {}

_NC_CACHE = []


def _mm(nc, out, lhsT, rhs, **kw):
    nc.tensor.matmul(out, lhsT, rhs, **kw)


def _build_nc():
    nc = bacc.Bacc(trn_type="TRN2", target_bir_lowering=False, debug=False)
    xT = nc.dram_tensor("xT", [C, T], bf16, kind="ExternalInput").ap()
    wqk = nc.dram_tensor("wqk", [8, 128, 1024], bf16, kind="ExternalInput").ap()
    wv = nc.dram_tensor("wv", [C, OG], bf16, kind="ExternalInput").ap()
    wo = nc.dram_tensor("wo", [OG, C], bf16, kind="ExternalInput").ap()
    masks = nc.dram_tensor("masks", [128, 1280], bf16, kind="ExternalInput").ap()
    ones_in = nc.dram_tensor("ones_in", [128, 128], f32r, kind="ExternalInput").ap()
    y = nc.dram_tensor("y", [T, C], f32, kind="ExternalOutput").ap()
    dbg = None
    if DEBUG:
        dbg = {
            "qt": nc.dram_tensor("dbg_qt", [128, 4, T], bf16, kind="ExternalOutput").ap(),
            "kt": nc.dram_tensor("dbg_kt", [128, 4, T], bf16, kind="ExternalOutput").ap(),
            "v": nc.dram_tensor("dbg_v", [128, NT, 4 * PAIR_W], bf16, kind="ExternalOutput").ap(),
            "attT": nc.dram_tensor("dbg_attT", [128, 4, T], bf16, kind="ExternalOutput").ap(),
        }

    with tile.TileContext(nc) as tc:
        _body(tc, nc, xT, wqk, wv, wo, masks, ones_in, y, dbg)
    nc.compile()
    return nc


def _drive(*gens):
    """Round-robin the generators until all are exhausted."""
    live = list(gens)
    while live:
        nxt = []
        for g in live:
            try:
                next(g)
                nxt.append(g)
            except StopIteration:
                pass
        live = nxt


def _body(tc, nc, xT, wqk, wv, wo, masks, ones_in, y, dbg):
    exp_f = mybir.ActivationFunctionType.Exp

    with (
        tc.tile_pool(name="persist", bufs=1) as persist,
        tc.tile_pool(name="wv_p", bufs=1) as wv_p,
        tc.tile_pool(name="xh_p", bufs=1) as xh_p,
        tc.tile_pool(name="wqk_p", bufs=1) as wqk_p,
        tc.tile_pool(name="mask_p", bufs=1) as mask_p,
        tc.tile_pool(name="ones_p", bufs=1) as ones_p,
        tc.tile_pool(name="wo_p", bufs=1) as wo_p,
        tc.tile_pool(name="pt_p", bufs=3) as pt_p,
        tc.tile_pool(name="sums_p", bufs=1) as sums_p,
        tc.tile_pool(name="bcast_p", bufs=1) as bcast_p,
        tc.tile_pool(name="yo_p", bufs=2) as yo_p,
        tc.tile_pool(name="big_ps", bufs=3, space="PSUM") as big_ps,
        tc.tile_pool(name="av_ps", bufs=2, space="PSUM") as av_ps,
    ):
        qt = persist.tile([128, 4, T], bf16)
        kt = persist.tile([128, 4, T], bf16)
        v_sb = persist.tile([128, NT, 4 * PAIR_W], bf16)
        attT = persist.tile([128, 4, T], bf16)

        mk = mask_p.tile([128, 1280], bf16)
        ones_sb = ones_p.tile([128, 128], f32r)
        wo_sb = wo_p.tile([128, 4, C], bf16)
        wv_sb = wv_p.tile([128, KT_C, OG], bf16)
        wts = {}

        # ================= projections (one half of T) =================
        # k-outer: each k-step of the contraction only needs xT k-slice k,
        # so matmuls start as soon as the first DMA lands.  Sub-waves hold a
        # single 2-bank psum tile so the shared pool stays fluid for the
        # interleaved attention chunks.
        def emit_half(half):
            t0 = half * (T // 2)
            xs = []
            for k in range(KT_C):
                xt = xh_p.tile(
                    [128, T // 2], bf16, tag=f"xh{k}", name=f"xh{half}_{k}"
                )
                xs.append(xt)

            def load_xh(k):
                nc.sync.dma_start(
                    xs[k][:], xT[k * 128 : (k + 1) * 128, t0 : t0 + T // 2]
                )

            if half == 0:
                # priority order: first k-slice + first-wave weights, then
                # the rest, then V weights / v_sb ones init / consts.
                load_xh(0)
                for m in range(4):
                    wt = wqk_p.tile([128, 1024], bf16, tag=f"wqk{m}", name=f"wt{m}")
                    nc.sync.dma_start(wt[:], wqk[m, :, :])
                    wts[m] = wt
                for k in range(1, KT_C):
                    load_xh(k)
                for m in range(4, 8):
                    wt = wqk_p.tile([128, 1024], bf16, tag=f"wqk{m}", name=f"wt{m}")
                    nc.sync.dma_start(wt[:], wqk[m, :, :])
                    wts[m] = wt
                for k in range(KT_C):
                    nc.sync.dma_start(wv_sb[:, k, :], wv[k * 128 : (k + 1) * 128, :])
                # v_sb ones/zeros bands: cols [64:129) of each pair are the
                # even/odd sums columns (64,65 = 1) + the odd-head zero block
                vz = v_sb[:].rearrange("p t (q w) -> p (t q) w", q=4, w=PAIR_W)
                nc.vector.memset(vz[:, :, 66:129], 0.0)
                nc.vector.memset(vz[:, :, 64:66], 1.0)
                nc.sync.dma_start(mk[:], masks[:])
                nc.sync.dma_start(ones_sb[:], ones_in[:])
                for k in range(4):
                    nc.sync.dma_start(wo_sb[:, k, :], wo[k * 128 : (k + 1) * 128, :])
            else:
                for k in range(KT_C):
                    load_xh(k)
            yield

            for wave in range(2):  # A: q (m 0..3), B: k (m 4..7)
                dst = qt if wave == 0 else kt
                for sub in range(4):
                    m = wave * 4 + sub
                    big = big_ps.tile(
                        [128, 1024], f32, tag="big", name=f"pw{half}_{m}"
                    )
                    for k in range(KT_C):
                        for n in range(2):
                            _mm(
                                nc,
                                big[:, n * 512 : (n + 1) * 512],
                                wts[m][:, k * 128 : (k + 1) * 128],
                                xs[k][:, n * 512 : (n + 1) * 512],
                                start=(k == 0),
                                stop=(k == KT_C - 1),
                            )
                    for n in range(2):
                        nc.vector.tensor_copy(
                            dst[:, sub, t0 + n * 512 : t0 + (n + 1) * 512],
                            big[:, n * 512 : (n + 1) * 512],
                        )
                    yield

            # V wave: out rows t, free = o (head-major)
            for sub in range(4):
                big = big_ps.tile(
                    [128, 1024], f32, tag="big", name=f"pv{half}_{sub}"
                )
                for k in range(KT_C):
                    for tli in range(2):
                        tl = sub * 2 + tli
                        _mm(
                            nc,
                            big[:, tli * 512 : (tli + 1) * 512],
                            xs[k][:, tl * 128 : (tl + 1) * 128],
                            wv_sb[:, k, :],
                            start=(k == 0),
                            stop=(k == KT_C - 1),
                        )
                for tli in range(2):
                    tl = sub * 2 + tli
                    tt = half * 8 + tl
                    ps = big[:, tli * 512 : (tli + 1) * 512]
                    src = ps.rearrange("p (h d) -> p h d", d=64)
                    dstv = v_sb[:, tt, :].rearrange("p (q w) -> p q w", w=PAIR_W)
                    nc.vector.tensor_copy(dstv[:, :, 0:64], src[:, 0::2, :])
                    nc.vector.tensor_copy(dstv[:, :, 129:193], src[:, 1::2, :])
                yield

        # ================= attention =================
        def head_ctx(hl):
            """Slices/layout facts for local head hl."""
            p0 = (hl % 2) * 64
            mt = hl // 2
            qrow = slice(p0, p0 + 64)
            vb0 = (hl // 2) * PAIR_W
            if hl % 2 == 0:
                vsl = (vb0, vb0 + 65)  # [V|1] -> rows 0..64
                srow, arow = 64, slice(0, 64)
            else:
                vsl = (vb0 + 65, vb0 + 193)  # [1|0*63|V] -> row 0 sums, 64..127 att
                srow, arow = 0, slice(64, 128)
            return p0, mt, qrow, vsl, srow, arow

        def do_chunk(j):
            ntk = 4 * j + 4
            ng = ntk // 2
            tq = slice(j * 512, (j + 1) * 512)
            for ha in range(0, HPG, 2):
                ctxs = [head_ctx(ha), head_ctx(ha + 1)]
                pts = {0: [None] * ng, 1: [None] * ng}

                def emit_pair(s, g):
                    _, mt, qrow, _, _, _ = ctxs[s]
                    # diagonal tiles only need tq >= tk: narrow the
                    # st/exp/av width (512/384/256/128) instead of masking
                    # fully-computed tiles.
                    geom = []  # per u: (tq_off, width, pt_col)
                    pcol = 0
                    for u in range(2):
                        tk = 2 * g + u
                        v = tk - 4 * j
                        off = 128 * v if v > 0 else 0
                        w = 512 - off
                        if u == 1 and pcol == 512:
                            pcol = 512  # second slot starts at bank 1
                        geom.append((off, w, pcol))
                        pcol = 512 if u == 0 and w == 512 else pcol + w
                    dg = 2 * g - 4 * j
                    ps = big_ps.tile(
                        [128, 1024], f32, tag="big", name=f"st_{j}_{ha}_{s}_{g}"
                    )
                    for u in range(2):
                        off, w, pc = geom[u]
                        tk = 2 * g + u
                        _mm(
                            nc,
                            ps[:, pc : pc + w],
                            kt[qrow, mt, tk * 128 : (tk + 1) * 128],
                            qt[qrow, mt, j * 512 + off : (j + 1) * 512],
                            start=True,
                            stop=True,
                        )
                    tot = geom[1][2] + geom[1][1]
                    pt = pt_p.tile([128, 1024], bf16, tag=f"pt{s}")
                    nc.scalar.activation(
                        pt[:, 0:tot], ps[:, 0:tot], exp_f, scale=0.125
                    )
                    if dg == 0:  # pair (4j, 4j+1): widths 512|384
                        nc.vector.tensor_mul(
                            pt[:, 0:896], pt[:, 0:896], mk[:, 0:896]
                        )
                    elif dg == 2:  # pair (4j+2, 4j+3): widths 256|128
                        nc.vector.tensor_mul(
                            pt[:, 0:384], pt[:, 0:384], mk[:, 896:1280]
                        )
                    pts[s][g] = (pt, geom)

                avs = [
                    av_ps.tile([128, 512], f32, tag="av", name=f"av{s}_{ha}_{j}")
                    for s in (0, 1)
                ]
                emit_pair(0, 0)
                emit_pair(1, 0)
                for g in range(ng):
                    if g + 1 < ng:
                        emit_pair(0, g + 1)
                        emit_pair(1, g + 1)
                    for u in range(2):
                        for s in (0, 1):
                            _, _, _, vsl, _, _ = ctxs[s]
                            pt, geom = pts[s][g]
                            off, w, pc = geom[u]
                            tk = 2 * g + u
                            _mm(
                                nc,
                                avs[s][0 : vsl[1] - vsl[0], off : off + w],
                                v_sb[:, tk, vsl[0] : vsl[1]],
                                pt[:, pc : pc + w],
                                start=(tk == 0),
                                stop=(tk == ntk - 1),
                            )

                for s in (0, 1):
                    _, mt, _, _, srow, arow = ctxs[s]
                    av = avs[s]
                    sums_sb = sums_p.tile([128, 512], f32r, tag=f"rc{s}")
                    nc.scalar.copy(
                        sums_sb[srow : srow + 1, :], av[srow : srow + 1, :]
                    )
                    bps = big_ps.tile(
                        [128, 1024], f32, tag="big", name=f"bps_{ha}_{j}_{s}"
                    )
                    _mm(
                        nc,
                        bps[:, 0:512],
                        ones_sb[srow : srow + 1, :],
                        sums_sb[srow : srow + 1, :],
                        start=True,
                        stop=True,
                    )
                    bc = bcast_p.tile([128, 512], f32, tag=f"bc{s}")
                    nc.vector.reciprocal_approx_fast(bc[:], bps[:, 0:512])
                    nc.vector.tensor_mul(
                        attT[arow, mt, tq], av[arow, :], bc[arow, :]
                    )
                yield

        def do_outproj_chunk(j):
            # y rows for tq chunk j: 4 t-tiles x 2 o-halves
            for tl in range(4):
                tt = 4 * j + tl
                yps = big_ps.tile([128, 1024], f32, tag="big", name=f"yps_{tt}")
                pso = [yps[:, 0:512], yps[:, 512:1024]]
                for k in range(4):
                    for o in range(2):
                        _mm(
                            nc,
                            pso[o],
                            attT[:, k, tt * 128 : (tt + 1) * 128],
                            wo_sb[:, k, o * 512 : (o + 1) * 512],
                            start=(k == 0),
                            stop=(k == 3),
                        )
                for o in range(2):
                    yo = yo_p.tile([128, 512], f32, tag="yo", name=f"yo_{tt}_{o}")
                    nc.vector.tensor_copy(yo[:], pso[o])
                    nc.sync.dma_start(
                        y[tt * 128 : (tt + 1) * 128, o * 512 : (o + 1) * 512],
                        yo[:],
                    )
                yield

        def chain(*gens):
            for g in gens:
                yield from g

        # phase A: half-0 projections (serial)
        for _ in emit_half(0):
            pass
        # phase B: chunk-0/1 attention interleaved with half-1 projections
        _drive(chain(do_chunk(0), do_chunk(1)), emit_half(1))
        # phase C: chunk-2 pairs interleaved with chunk-0/1 out-projections
        _drive(do_chunk(2), chain(do_outproj_chunk(0), do_outproj_chunk(1)))
        # phase D: chunk-3 pairs interleaved with chunk-2 out-projection
        _drive(do_chunk(3), do_outproj_chunk(2))
        # phase E: chunk-3 out-projection
        for _ in do_outproj_chunk(3):
            pass

        if dbg is not None:
            for mm_ in range(4):
                nc.sync.dma_start(dbg["qt"][:, mm_, :], qt[:, mm_, :])
                nc.sync.dma_start(dbg["kt"][:, mm_, :], kt[:, mm_, :])
                nc.sync.dma_start(dbg["attT"][:, mm_, :], attT[:, mm_, :])
            for tt_ in range(NT):
                nc.sync.dma_start(dbg["v"][:, tt_, :], v_sb[:, tt_, :])


def _round_fp32r(a):
    """Round fp32 to the fp32r grid (11 mantissa bits; low 12 bits zero), RNE."""
    u = np.ascontiguousarray(a, dtype=np.float32).view(np.uint32)
    lsb = (u >> 12) & 1
    out = ((u + 0x7FF + lsb) & 0xFFFFF000).astype(np.uint32)
    return out.view(np.float32)


def _host_prep(x, w_qkv, w_out):
    xT_all = np.ascontiguousarray(x.transpose(0, 2, 1)).astype(BF16)
    # packed diagonal masks, all variant-0 (keep iff tq_local >= tk_local):
    # [0:512) pair1-u0 w=512, [512:896) pair1-u1 w=384,
    # [896:1152) pair2-u0 w=256, [1152:1280) pair2-u1 w=128
    tk_l = np.arange(128)[:, None]
    m0 = (np.arange(512)[None, :] >= tk_l).astype(BF16)
    masks = np.concatenate([m0, m0[:, :384], m0[:, :256], m0[:, :128]], axis=1)

    per_group = []
    for g in range(HG):
        wq = w_qkv[g * OG : (g + 1) * OG]
        wk = w_qkv[C + g * OG : C + (g + 1) * OG]
        wvg = w_qkv[2 * C + g * OG : 2 * C + (g + 1) * OG]
        wqkT = np.concatenate([wq, wk], axis=0).T  # (C, 1024)
        # wqk_r[m, p, k*128+j] = wqkT[k*128+p, m*128+j]
        wqk_r = np.ascontiguousarray(
            wqkT.reshape(8, 128, 8, 128).transpose(2, 1, 0, 3).reshape(8, 128, 1024)
        ).astype(BF16)
        wv_t = np.ascontiguousarray(wvg.T).astype(BF16)  # (C, 512)
        wo_t = np.ascontiguousarray(w_out.T[g * OG : (g + 1) * OG]).astype(
            BF16
        )  # (512, C)
        per_group.append((wqk_r, wv_t, wo_t))
    ones_in = np.ones((128, 128), np.float32)
    return xT_all, masks, ones_in, per_group


def kernel(x, w_qkv, w_out):
    x = np.asarray(x)
    w_qkv = np.asarray(w_qkv)
    w_out = np.asarray(w_out)
    xT_all, masks, ones_in, per_group = _host_prep(x, w_qkv, w_out)

    if not _NC_CACHE:
        _NC_CACHE.append(_build_nc())
    nc = _NC_CACHE[0]

    in_maps = []
    for core in range(8):
        b, g = core // 2, core % 2
        wqk_r, wv_t, wo_t = per_group[g]
        in_maps.append(
            {"xT": xT_all[b], "wqk": wqk_r, "wv": wv_t, "wo": wo_t, "masks": masks,
             "ones_in": ones_in}
        )

    res = bass_utils.run_bass_kernel_spmd(
        nc, in_maps, core_ids=list(range(8)), trace=TRACE
    )
    LAST_RUN["res"] = res

    y = np.empty((B, T, C), np.float32)
    for b in range(B):
        y[b] = res.results[2 * b]["y"] + res.results[2 * b + 1]["y"]
    return y
